# revision 6
# baseline (speedup 1.0000x reference)
"""Trainium2 Bass kernel for nn_BoxFilter: 21x21 all-ones box filter with
circular (wrap) padding over x of shape (8, 1, 2048, 2048) fp32.

Strategy (data-parallel, one image per NeuronCore, 8 cores):
  The 21x21 ones kernel is separable: out = vertical_box21(horizontal_box21(x)).
  The whole on-chip pipeline runs in fp16 (values are O(30), fp16's 10-bit
  mantissa gives ~4e-4 end-to-end rel error vs the 2e-2 gate):
  the host casts x to fp16 before upload, which also halves input DMA bytes.

  Per core, per 128-row tile (tile rows are shifted by -10 so each tile holds
  the halo rows its output strip needs):
    1. DMA the fp16 tile rows into SBUF at xe[:, 31:], with 10 wrap columns
       on each side and 21 zero columns in front (xe width 21+10+2048+10).
    2. One DVE tensor_tensor_scan computes the horizontal box sum via the
       running-window recurrence
           state_t = (xe[21+t] + state_{t-1}) - xe[t]
       (leading 21 zero columns build the first window; the scan state is
       fp32 regardless of operand dtype), writing y in fp16.
    3. TensorE: vertical box sum as banded-ones matmuls in fp16 (1 col/cycle
       vs ~3 for fp32). For output strip r:
       out_strip = S1.T @ y_r + S2.T @ y_{r+1}  where S1 is a 128x128 band
       (1 iff 0 <= p-m <= 20) and S2 is 128x128 (1 iff m-p >= 108, nonzero
       only in rows < 20; kept full so every LDWEIGHTS is a uniform 128x128
       tile, which walrus ldw-opt requires for 2-byte weights).
    4. PSUM -> SBUF staging copies alternate between ScalarE (ACT) and the
       otherwise-idle Pool engine; each engine's strips drain to HBM on its
       own DMA path (ACT HWDGE ring / Pool SWDGE ring), with the Sync ring
       carrying the fp16 inputs - three DMA paths in parallel.

  H-wrap is handled by tile indexing mod 16 (strip 15 reuses tile 0's y);
  W-wrap by the 10 wrap columns of xe.
"""

import sys
import types

import numpy as np

for _p in ("/opt/trn_rl_repo",):
    if _p not in sys.path:
        sys.path.append(_p)

import concourse.bass as bass
import concourse.bacc as bacc
import concourse.mybir as mybir
from concourse.tile import TileContext
import concourse.bass_utils as bass_utils

# ---- problem constants (hardcoded per harness contract) ----
B = 8          # batch == number of cores
H = 2048
W = 2048
R = 10         # box filter half-width (both axes)
WIN = 2 * R + 1
P = 128        # partitions

f32 = mybir.dt.float32
f16 = mybir.dt.float16

import os as _os

LDW_OPT = _os.environ.get("BOXF_LDW_OPT", "1") == "1"
POOL_COPY = _os.environ.get("BOXF_POOL_COPY", "1") == "1"
POOL_OUT = _os.environ.get("BOXF_POOL_OUT", "1") == "1"
XE_BUFS = int(_os.environ.get("BOXF_XE_BUFS", "6"))
ST_BUFS = int(_os.environ.get("BOXF_ST_BUFS", "4"))
PSUM_BUFS = 2   # full-strip tiles, 4 banks each


def _patch_walrus_ldw_opt():
    """Enable walrus LDWEIGHTS dedup: consecutive matmuls reusing the same
    stationary skip the reload."""
    if getattr(bass_utils, "_ldw_patched", False):
        return
    orig = bass_utils.run_command

    def run_command2(argv, **kw):
        argv = [
            "--enable-ldw-opt=true" if a == "--enable-ldw-opt=false" else a
            for a in argv
        ]
        return orig(argv, **kw)

    bass_utils.run_command = run_command2
    bass_utils._ldw_patched = True


def _band_matrices(scale: float):
    """Stationary (lhsT) band matrices for the vertical pass (fp16)."""
    p = np.arange(P)[:, None]
    m = np.arange(P)[None, :]
    s1 = ((p - m >= 0) & (p - m <= 2 * R)).astype(np.float32) * scale
    s2 = (m - p >= 108).astype(np.float32) * scale
    return s1.astype(np.float16), s2.astype(np.float16)


def _build_bass(h: int, w: int):
    """Build the per-core Bass program for an h x w image."""
    salt = _os.environ.get("BOXF_SALT", "")
    nt = h // P
    xw = WIN + R + w + R    # 21 zeros | 10 wrap | w | 10 wrap  = w + 41
    yw = 2 * R + w          # scan output width; y[:, 20+j] is the box sum
    nbanks = (w + 511) // 512

    nc = bacc.Bacc("TRN2", target_bir_lowering=False, debug=False)

    x_in = nc.dram_tensor("x", [h, w], f16, kind="ExternalInput")
    s1_in = nc.dram_tensor("s1", [P, P], f16, kind="ExternalInput")
    s2_in = nc.dram_tensor("s2", [P, P], f16, kind="ExternalInput")
    out = nc.dram_tensor("out", [h, w], f32, kind="ExternalOutput")

    with TileContext(nc) as tc:
        with (
            tc.tile_pool(name="const" + salt, bufs=1) as const_pool,
            tc.tile_pool(name="work", bufs=1) as work,
            tc.tile_pool(name="psum", bufs=PSUM_BUFS, space="PSUM") as psum_pool,
        ):
            s1 = const_pool.tile([P, P], f16, tag="s1")
            nc.sync.dma_start(out=s1[:], in_=s1_in[:])
            s2 = const_pool.tile([P, P], f16, tag="s2")
            nc.sync.dma_start(out=s2[:], in_=s2_in[:])

            y_tiles = [None] * nt

            def make_tile(t):
                """Tiles hold input rows [128t - 10, 128t + 118) mod h."""
                xe = work.tile([P, xw], f16, tag="xe", bufs=XE_BUFS)
                r0 = (P * t - R) % h
                col0 = WIN + R  # where x columns start inside xe
                # first two tiles ride the (otherwise idle-at-start) ACT ring
                # so both HWDGE rings stream inputs in parallel and the scan
                # chain's head starts as early as possible
                dma = nc.scalar if t < 2 else nc.sync
                if r0 + P <= h:
                    dma.dma_start(
                        out=xe[:, col0 : col0 + w], in_=x_in[r0 : r0 + P, :]
                    )
                else:
                    k = h - r0
                    dma.dma_start(out=xe[:k, col0 : col0 + w], in_=x_in[r0:h, :])
                    dma.dma_start(
                        out=xe[k:P, col0 : col0 + w], in_=x_in[0 : P - k, :]
                    )
                # leading zeros for the window build-up: the zero columns are
                # never overwritten, so each xe buffer only needs them once
                if t < XE_BUFS:
                    nc.vector.memset(xe[:, 0:WIN], 0.0)
                # wrap columns (on DVE: these also absorb the DMA-completion
                # waits, since the scan's ISA struct cannot carry sync waits)
                nc.vector.tensor_copy(
                    out=xe[:, WIN : WIN + R], in_=xe[:, col0 + w - R : col0 + w]
                )
                nc.vector.tensor_copy(
                    out=xe[:, col0 + w : xw], in_=xe[:, col0 : col0 + R]
                )

                # y tiles are written once and stay resident all kernel
                y = work.tile([P, yw], f16, tag=f"y{t}", bufs=1)
                # running-window recurrence: state = (xe[21+t] + state) - xe[t]
                nc.vector.tensor_tensor_scan(
                    out=y[:, 0:yw],
                    data0=xe[:, WIN : WIN + yw],
                    data1=xe[:, 0:yw],
                    initial=0.0,
                    op0=mybir.AluOpType.add,
                    op1=mybir.AluOpType.subtract,
                )
                y_tiles[t] = y

            def make_strip(r):
                """Output rows [128r, 128r + 128)."""
                y_cur = y_tiles[r]
                y_nxt = y_tiles[(r + 1) % nt]
                psum = psum_pool.tile([P, w], f32, tag="psum")
                for b in range(nbanks):
                    lo, hi = b * 512, min((b + 1) * 512, w)
                    nc.tensor.matmul(
                        psum[:, lo:hi],
                        lhsT=s1[:],
                        rhs=y_cur[:, 2 * R + lo : 2 * R + hi],
                        start=True,
                        stop=False,
                    )
                for b in range(nbanks):
                    lo, hi = b * 512, min((b + 1) * 512, w)
                    nc.tensor.matmul(
                        psum[:, lo:hi],
                        lhsT=s2[:],
                        rhs=y_nxt[:, 2 * R + lo : 2 * R + hi],
                        start=False,
                        stop=True,
                    )
                st = work.tile([P, w], f32, tag="st", bufs=ST_BUFS)
                # ACT drains PSUM (Pool cannot access PSUM); the HBM
                # writeback alternates between the ACT HWDGE ring and the
                # Pool SWDGE ring so output traffic uses two DMA paths
                nc.scalar.copy(st[:], psum[:])
                dma = nc.gpsimd if (POOL_OUT and r % 2 == 1) else nc.scalar
                dma.dma_start(out=out[P * r : P * (r + 1), :], in_=st[:])

            make_tile(0)
            for t in range(1, nt):
                make_tile(t)
                make_strip(t - 1)
            make_strip(nt - 1)

    nc.finalize()
    return nc


_BUILD_CACHE = {}


def _get_bass(h, w):
    key = (h, w, POOL_COPY, POOL_OUT, XE_BUFS, ST_BUFS)
    if key not in _BUILD_CACHE:
        _BUILD_CACHE[key] = _build_bass(h, w)
    return _BUILD_CACHE[key]


def _enable_ntff_tracing():
    """Harness-only: register the axon NTFF profile hook and stub the
    artifact upload (no bucket creds in this container)."""
    import antenv

    if not hasattr(antenv, "axon_hooks"):
        mod = types.ModuleType("antenv.axon_hooks")
        _hook = [None]
        mod.set_axon_ntff_profile_hook = lambda hk: _hook.__setitem__(0, hk)
        mod.get_axon_ntff_profile_hook = lambda: _hook[0]
        sys.modules["antenv.axon_hooks"] = mod
        antenv.axon_hooks = mod
    from trn_agent_boot.trn_boot import _ntff_profile_via_ctypes

    hook = _ntff_profile_via_ctypes("/opt/axon/libaxon_pjrt.so")
    if hook is not None:
        antenv.axon_hooks.set_axon_ntff_profile_hook(hook)
    bass_utils.upload_artifacts = lambda tmpdir: tmpdir


def run_hw(x, kernelx, trace=False):
    """Run the box filter on 8 NeuronCores. Returns (out, BassKernelResults)."""
    x = np.asarray(x)
    scale = float(np.asarray(kernelx).flat[0])
    s1, s2 = _band_matrices(scale)

    if trace:
        _enable_ntff_tracing()
    if LDW_OPT:
        _patch_walrus_ldw_opt()

    nc = _get_bass(H, W)
    x16 = np.ascontiguousarray(x.astype(np.float16))
    in_maps = [
        {"x": x16[i, 0], "s1": s1, "s2": s2} for i in range(B)
    ]
    r = bass_utils.run_bass_kernel_spmd(nc, in_maps, core_ids=list(range(B)),
                                        trace=trace)
    outs = np.stack([r.results[i]["out"] for i in range(B)])[:, None]
    return outs.astype(np.float32, copy=False), r


def _fallback_numpy(x, kernelx):
    """Exact (slow) path for a non-uniform kernel; never hit for the graded
    setup_inputs (all-ones kernel)."""
    x64 = np.asarray(x, dtype=np.float64)[:, 0]
    k = np.asarray(kernelx, dtype=np.float64)[0, 0]
    out = np.zeros_like(x64)
    for a in range(k.shape[0]):
        for b_ in range(k.shape[1]):
            if k[a, b_] == 0.0:
                continue
            out += k[a, b_] * np.roll(
                np.roll(x64, R - a, axis=1), R - b_, axis=2
            )
    return out[:, None].astype(np.float32)


def kernel(x, kernelx):
    kx = np.asarray(kernelx)
    if kx.size and not np.all(kx == kx.flat[0]):
        return _fallback_numpy(x, kernelx)
    out, _ = run_hw(x, kernelx, trace=False)
    return out


# revision 7
# speedup vs baseline: 1.0953x; 1.0953x over previous
"""Trainium2 Bass kernel for nn_BoxFilter: 21x21 all-ones box filter with
circular (wrap) padding over x of shape (8, 1, 2048, 2048) fp32.

Strategy (data-parallel, one image per NeuronCore, 8 cores):
  The 21x21 ones kernel is separable: out = vertical_box21(horizontal_box21(x)).

  Measured HW rates drove the design:
    - DVE tensor_tensor_scan: 2.15 ns/elem for fp32 OR bf16 inputs (fp16
      inputs are 1.5x slower); output dtype is free. The scan is the one
      op only DVE can do -> DVE runs ONLY scans (71 us, the wall).
    - Warm fp16/bf16 matmul: 379 ns / 512 cols + ~100 ns LDWEIGHTS.
    - Pool (gpsimd) does SBUF-only vector ops ~2.2 ns/elem, no PSUM, and
      issues SWDGE DMAs -> it takes the wrap-column copies + half the
      output DMA issues off the critical engines.
    - ACT drains PSUM fp32 -> fp16 at ~1.4 ns/elem.
    - All DMA queues share ~420 GB/s; bf16 input + fp16 output = 16.8 MB
      per core (40 us floor).

  Per core, per 128-row tile (rows shifted by -10 so each tile holds the
  halo rows its output strip needs):
    1. DMA the bf16 tile rows into SBUF at xe[:, 31:] (xe row layout:
       21 zero cols | 10 W-wrap | 2048 | 10 W-wrap).
    2. Pool fills the wrap columns (and the zero head, once per buffer).
    3. One DVE tensor_tensor_scan computes the horizontal box sum via
           state_t = (xe[21+t] + state_{t-1}) - xe[t]
       (fp32 internal state; bf16 in/out), writing y in bf16.
    4. TensorE: vertical box sum as banded-ones bf16 matmuls. For strip r:
       out_strip = S1.T @ y_r + S2.T @ y_{r+1} with S1[p,m] = 1 iff
       0 <= p-m <= 20 and S2[p,m] = 1 iff m-p >= 108 (full 128x128).
    5. ACT drains PSUM to SBUF casting fp32 -> fp16; strips go to HBM as
       fp16 (host upcasts) on alternating DMA paths (ACT HWDGE ring /
       Pool SWDGE ring), inputs ride the Sync ring.

  End-to-end rel error ~3e-3 vs the 2e-2 gate (bf16 input quantization
  dominates; the scan state and PSUM accumulation are fp32).

  H-wrap is handled by tile indexing mod 16 (strip 15 reuses tile 0's y);
  W-wrap by the 10 wrap columns of xe.
"""

import sys
import types

import numpy as np
import ml_dtypes

for _p in ("/opt/trn_rl_repo",):
    if _p not in sys.path:
        sys.path.append(_p)

import concourse.bass as bass
import concourse.bacc as bacc
import concourse.mybir as mybir
from concourse.tile import TileContext
import concourse.bass_utils as bass_utils

# ---- problem constants (hardcoded per harness contract) ----
B = 8          # batch == number of cores
H = 2048
W = 2048
R = 10         # box filter half-width (both axes)
WIN = 2 * R + 1
P = 128        # partitions

f32 = mybir.dt.float32
f16 = mybir.dt.float16
bf16 = mybir.dt.bfloat16

import os as _os

POOL_PRE = _os.environ.get("BOXF_POOL_PRE", "1") == "1"    # wrap copies on Pool
POOL_OUT = _os.environ.get("BOXF_POOL_OUT", "1") == "1"    # odd strips out via SWDGE
OUT_LOOKAHEAD = int(_os.environ.get("BOXF_OUT_LOOKAHEAD", "4"))
XE_BUFS = int(_os.environ.get("BOXF_XE_BUFS", "6"))
ST_BUFS = int(_os.environ.get("BOXF_ST_BUFS", "4"))
PSUM_BUFS = 2   # full-strip tiles, 4 banks each


def _band_matrices(scale: float):
    """Stationary (lhsT) band matrices for the vertical pass (bf16)."""
    p = np.arange(P)[:, None]
    m = np.arange(P)[None, :]
    s1 = ((p - m >= 0) & (p - m <= 2 * R)).astype(np.float32) * scale
    s2 = (m - p >= 108).astype(np.float32) * scale
    return s1.astype(ml_dtypes.bfloat16), s2.astype(ml_dtypes.bfloat16)


def _build_bass(h: int, w: int):
    """Build the per-core Bass program for an h x w image."""
    salt = _os.environ.get("BOXF_SALT", "")
    nt = h // P
    xw = WIN + R + w + R    # 21 zeros | 10 wrap | w | 10 wrap  = w + 41
    yw = 2 * R + w          # scan output width; y[:, 20+j] is the box sum
    nbanks = (w + 511) // 512

    nc = bacc.Bacc("TRN2", target_bir_lowering=False, debug=False)

    x_in = nc.dram_tensor("x", [h, w], bf16, kind="ExternalInput")
    s1_in = nc.dram_tensor("s1", [P, P], bf16, kind="ExternalInput")
    s2_in = nc.dram_tensor("s2", [P, P], bf16, kind="ExternalInput")
    out = nc.dram_tensor("out", [h, w], f16, kind="ExternalOutput")

    with TileContext(nc) as tc:
        with (
            tc.tile_pool(name="const" + salt, bufs=1) as const_pool,
            tc.tile_pool(name="work", bufs=1) as work,
            tc.tile_pool(name="psum", bufs=PSUM_BUFS, space="PSUM") as psum_pool,
        ):
            s1 = const_pool.tile([P, P], bf16, tag="s1")
            nc.sync.dma_start(out=s1[:], in_=s1_in[:])
            s2 = const_pool.tile([P, P], bf16, tag="s2")
            nc.sync.dma_start(out=s2[:], in_=s2_in[:])

            y_tiles = [None] * nt
            st_tiles = [None] * nt

            pre = nc.gpsimd if POOL_PRE else nc.vector

            def make_tile(t):
                """Tiles hold input rows [128t - 10, 128t + 118) mod h."""
                xe = work.tile([P, xw], bf16, tag="xe", bufs=XE_BUFS)
                r0 = (P * t - R) % h
                col0 = WIN + R  # where x columns start inside xe
                # first two tiles ride the (otherwise idle-at-start) ACT ring
                # so both HWDGE rings stream inputs in parallel and the scan
                # chain's head starts as early as possible
                dma = nc.scalar if t < 2 else nc.sync
                if r0 + P <= h:
                    dma.dma_start(
                        out=xe[:, col0 : col0 + w], in_=x_in[r0 : r0 + P, :]
                    )
                else:
                    k = h - r0
                    dma.dma_start(out=xe[:k, col0 : col0 + w], in_=x_in[r0:h, :])
                    dma.dma_start(
                        out=xe[k:P, col0 : col0 + w], in_=x_in[0 : P - k, :]
                    )
                # leading zeros for the window build-up: the zero columns are
                # never overwritten, so each xe buffer only needs them once
                if t < XE_BUFS:
                    pre.memset(xe[:, 0:WIN], 0.0)
                # W-wrap columns (on Pool: keeps DVE scan-only)
                pre.tensor_copy(
                    out=xe[:, WIN : WIN + R], in_=xe[:, col0 + w - R : col0 + w]
                )
                pre.tensor_copy(
                    out=xe[:, col0 + w : xw], in_=xe[:, col0 : col0 + R]
                )

                # y tiles are written once and stay resident all kernel
                y = work.tile([P, yw], bf16, tag=f"y{t}", bufs=1)
                # running-window recurrence: state = (xe[21+t] + state) - xe[t]
                nc.vector.tensor_tensor_scan(
                    out=y[:, 0:yw],
                    data0=xe[:, WIN : WIN + yw],
                    data1=xe[:, 0:yw],
                    initial=0.0,
                    op0=mybir.AluOpType.add,
                    op1=mybir.AluOpType.subtract,
                )
                y_tiles[t] = y

            def make_strip(r):
                """Output rows [128r, 128r + 128): matmuls + ACT drain."""
                y_cur = y_tiles[r]
                y_nxt = y_tiles[(r + 1) % nt]
                psum = psum_pool.tile([P, w], f32, tag="psum")
                for b in range(nbanks):
                    lo, hi = b * 512, min((b + 1) * 512, w)
                    nc.tensor.matmul(
                        psum[:, lo:hi],
                        lhsT=s1[:],
                        rhs=y_cur[:, 2 * R + lo : 2 * R + hi],
                        start=True,
                        stop=False,
                    )
                for b in range(nbanks):
                    lo, hi = b * 512, min((b + 1) * 512, w)
                    nc.tensor.matmul(
                        psum[:, lo:hi],
                        lhsT=s2[:],
                        rhs=y_nxt[:, 2 * R + lo : 2 * R + hi],
                        start=False,
                        stop=True,
                    )
                st = work.tile([P, w], f16, tag="st", bufs=ST_BUFS)
                nc.scalar.copy(st[:], psum[:])
                st_tiles[r] = st
                # even strips: ACT issues its own writeback right away
                if not (POOL_OUT and r % 2 == 1):
                    nc.scalar.dma_start(out=out[P * r : P * (r + 1), :], in_=st[:])

            def make_strip_out(r):
                """Odd strips drain on the Pool SWDGE ring, issued a few
                tiles late so the wait never stalls Pool's wrap copies."""
                if POOL_OUT and r % 2 == 1:
                    nc.gpsimd.dma_start(
                        out=out[P * r : P * (r + 1), :], in_=st_tiles[r][:]
                    )

            make_tile(0)
            for t in range(1, nt):
                make_tile(t)
                make_strip(t - 1)
                if t - 1 - OUT_LOOKAHEAD >= 0:
                    make_strip_out(t - 1 - OUT_LOOKAHEAD)
            make_strip(nt - 1)
            for r in range(nt - OUT_LOOKAHEAD - 1, nt):
                if r >= 0:
                    make_strip_out(r)

    nc.finalize()
    return nc


_BUILD_CACHE = {}


def _get_bass(h, w):
    key = (h, w, POOL_PRE, POOL_OUT, XE_BUFS, ST_BUFS, OUT_LOOKAHEAD)
    if key not in _BUILD_CACHE:
        _BUILD_CACHE[key] = _build_bass(h, w)
    return _BUILD_CACHE[key]


def _enable_ntff_tracing():
    """Harness-only: register the axon NTFF profile hook and stub the
    artifact upload (no bucket creds in this container)."""
    import antenv

    if not hasattr(antenv, "axon_hooks"):
        mod = types.ModuleType("antenv.axon_hooks")
        _hook = [None]
        mod.set_axon_ntff_profile_hook = lambda hk: _hook.__setitem__(0, hk)
        mod.get_axon_ntff_profile_hook = lambda: _hook[0]
        sys.modules["antenv.axon_hooks"] = mod
        antenv.axon_hooks = mod
    from trn_agent_boot.trn_boot import _ntff_profile_via_ctypes

    hook = _ntff_profile_via_ctypes("/opt/axon/libaxon_pjrt.so")
    if hook is not None:
        antenv.axon_hooks.set_axon_ntff_profile_hook(hook)
    bass_utils.upload_artifacts = lambda tmpdir: tmpdir


def run_hw(x, kernelx, trace=False):
    """Run the box filter on 8 NeuronCores. Returns (out, BassKernelResults)."""
    x = np.asarray(x)
    scale = float(np.asarray(kernelx).flat[0])
    s1, s2 = _band_matrices(scale)

    if trace:
        _enable_ntff_tracing()

    nc = _get_bass(H, W)
    xb = np.ascontiguousarray(x.astype(ml_dtypes.bfloat16))
    in_maps = [
        {"x": xb[i, 0], "s1": s1, "s2": s2} for i in range(B)
    ]
    r = bass_utils.run_bass_kernel_spmd(nc, in_maps, core_ids=list(range(B)),
                                        trace=trace)
    outs = np.stack([np.asarray(r.results[i]["out"]) for i in range(B)])[:, None]
    return outs.astype(np.float32), r


def _fallback_numpy(x, kernelx):
    """Exact (slow) path for a non-uniform kernel; never hit for the graded
    setup_inputs (all-ones kernel)."""
    x64 = np.asarray(x, dtype=np.float64)[:, 0]
    k = np.asarray(kernelx, dtype=np.float64)[0, 0]
    out = np.zeros_like(x64)
    for a in range(k.shape[0]):
        for b_ in range(k.shape[1]):
            if k[a, b_] == 0.0:
                continue
            out += k[a, b_] * np.roll(
                np.roll(x64, R - a, axis=1), R - b_, axis=2
            )
    return out[:, None].astype(np.float32)


def kernel(x, kernelx):
    kx = np.asarray(kernelx)
    if kx.size and not np.all(kx == kx.flat[0]):
        return _fallback_numpy(x, kernelx)
    out, _ = run_hw(x, kernelx, trace=False)
    return out


# revision 8
# speedup vs baseline: 1.1074x; 1.0110x over previous
"""Trainium2 Bass kernel for nn_BoxFilter: 21x21 all-ones box filter with
circular (wrap) padding over x of shape (8, 1, 2048, 2048) fp32.

Strategy (data-parallel, one image per NeuronCore, 8 cores):
  The 21x21 ones kernel is separable: out = vertical_box21(horizontal_box21(x)).

  Measured HW rates drove the design:
    - DVE tensor_tensor_scan: 2.15 ns/elem for fp32 OR bf16 inputs (fp16
      inputs are 1.5x slower); output dtype is free. The scan is the one
      op only DVE can do -> DVE runs ONLY scans (71 us, the wall).
    - Warm fp16/bf16 matmul: 379 ns / 512 cols + ~100 ns LDWEIGHTS.
    - Pool (gpsimd) does SBUF-only vector ops ~2.2 ns/elem, no PSUM, and
      issues SWDGE DMAs -> it takes the wrap-column copies + half the
      output DMA issues off the critical engines.
    - ACT drains PSUM fp32 -> fp16 at ~1.4 ns/elem.
    - All DMA queues share ~420 GB/s; bf16 input + fp16 output = 16.8 MB
      per core (40 us floor).

  Per core, per 128-row tile (rows shifted by -10 so each tile holds the
  halo rows its output strip needs):
    1. DMA the bf16 tile rows into SBUF at xe[:, 31:] (xe row layout:
       21 zero cols | 10 W-wrap | 2048 | 10 W-wrap).
    2. Pool fills the wrap columns (and the zero head, once per buffer).
    3. One DVE tensor_tensor_scan computes the horizontal box sum via
           state_t = (xe[21+t] + state_{t-1}) - xe[t]
       (fp32 internal state; bf16 in/out), writing y in bf16.
    4. TensorE: vertical box sum as banded-ones bf16 matmuls. For strip r:
       out_strip = S1.T @ y_r + S2.T @ y_{r+1} with S1[p,m] = 1 iff
       0 <= p-m <= 20 and S2[p,m] = 1 iff m-p >= 108 (full 128x128).
    5. ACT drains PSUM to SBUF casting fp32 -> fp16; strips go to HBM as
       fp16 (host upcasts) on alternating DMA paths (ACT HWDGE ring /
       Pool SWDGE ring), inputs ride the Sync ring.

  End-to-end rel error ~3e-3 vs the 2e-2 gate (bf16 input quantization
  dominates; the scan state and PSUM accumulation are fp32).

  H-wrap is handled by tile indexing mod 16 (strip 15 reuses tile 0's y);
  W-wrap by the 10 wrap columns of xe.
"""

import sys
import types

import numpy as np
import ml_dtypes

for _p in ("/opt/trn_rl_repo",):
    if _p not in sys.path:
        sys.path.append(_p)

import concourse.bass as bass
import concourse.bacc as bacc
import concourse.mybir as mybir
from concourse.tile import TileContext
import concourse.bass_utils as bass_utils

# ---- problem constants (hardcoded per harness contract) ----
B = 8          # batch == number of cores
H = 2048
W = 2048
R = 10         # box filter half-width (both axes)
WIN = 2 * R + 1
P = 128        # partitions

f32 = mybir.dt.float32
f16 = mybir.dt.float16
bf16 = mybir.dt.bfloat16

import os as _os

PRE_ENG = _os.environ.get("BOXF_PRE_ENG", "scalar")        # wrap-copy engine
POOL_OUT = _os.environ.get("BOXF_POOL_OUT", "1") == "1"    # odd strips out via SWDGE
OUT_LOOKAHEAD = int(_os.environ.get("BOXF_OUT_LOOKAHEAD", "4"))
XE_BUFS = int(_os.environ.get("BOXF_XE_BUFS", "8"))
ST_BUFS = int(_os.environ.get("BOXF_ST_BUFS", "4"))
PSUM_BUFS = 2   # full-strip tiles, 4 banks each


def _band_matrices(scale: float):
    """Stationary (lhsT) band matrices for the vertical pass (bf16)."""
    p = np.arange(P)[:, None]
    m = np.arange(P)[None, :]
    s1 = ((p - m >= 0) & (p - m <= 2 * R)).astype(np.float32) * scale
    s2 = (m - p >= 108).astype(np.float32) * scale
    return s1.astype(ml_dtypes.bfloat16), s2.astype(ml_dtypes.bfloat16)


def _build_bass(h: int, w: int):
    """Build the per-core Bass program for an h x w image."""
    salt = _os.environ.get("BOXF_SALT", "")
    nt = h // P
    xw = WIN + R + w + R    # 21 zeros | 10 wrap | w | 10 wrap  = w + 41
    yw = 2 * R + w          # scan output width; y[:, 20+j] is the box sum
    nbanks = (w + 511) // 512

    nc = bacc.Bacc("TRN2", target_bir_lowering=False, debug=False)

    x_in = nc.dram_tensor("x", [h, w], bf16, kind="ExternalInput")
    s1_in = nc.dram_tensor("s1", [P, P], bf16, kind="ExternalInput")
    s2_in = nc.dram_tensor("s2", [P, P], bf16, kind="ExternalInput")
    out = nc.dram_tensor("out", [h, w], f16, kind="ExternalOutput")

    with TileContext(nc) as tc:
        with (
            tc.tile_pool(name="const" + salt, bufs=1) as const_pool,
            tc.tile_pool(name="work", bufs=1) as work,
            tc.tile_pool(name="psum", bufs=PSUM_BUFS, space="PSUM") as psum_pool,
        ):
            s1 = const_pool.tile([P, P], bf16, tag="s1")
            nc.sync.dma_start(out=s1[:], in_=s1_in[:])
            s2 = const_pool.tile([P, P], bf16, tag="s2")
            nc.sync.dma_start(out=s2[:], in_=s2_in[:])

            y_tiles = [None] * nt
            st_tiles = [None] * nt

            pre = {"scalar": nc.scalar, "gpsimd": nc.gpsimd,
                   "vector": nc.vector}[PRE_ENG]

            def make_tile(t):
                """Tiles hold input rows [128t - 10, 128t + 118) mod h."""
                xe = work.tile([P, xw], bf16, tag="xe", bufs=XE_BUFS)
                r0 = (P * t - R) % h
                col0 = WIN + R  # where x columns start inside xe
                # all inputs ride the Sync ring: it reliably streams at
                # ~200+ GB/s from t~9us, while mixing inputs onto the ACT
                # ring proved to crawl at startup
                dma = nc.sync
                if r0 + P <= h:
                    dma.dma_start(
                        out=xe[:, col0 : col0 + w], in_=x_in[r0 : r0 + P, :]
                    )
                else:
                    k = h - r0
                    dma.dma_start(out=xe[:k, col0 : col0 + w], in_=x_in[r0:h, :])
                    dma.dma_start(
                        out=xe[k:P, col0 : col0 + w], in_=x_in[0 : P - k, :]
                    )
                # leading zeros for the window build-up: the zero columns are
                # never overwritten, so each xe buffer only needs them once
                if t < XE_BUFS:
                    nc.gpsimd.memset(xe[:, 0:WIN], 0.0)
                # W-wrap columns (kept off DVE so it only scans)
                if PRE_ENG == "scalar":
                    pre.copy(xe[:, WIN : WIN + R], xe[:, col0 + w - R : col0 + w])
                    pre.copy(xe[:, col0 + w : xw], xe[:, col0 : col0 + R])
                else:
                    pre.tensor_copy(
                        out=xe[:, WIN : WIN + R],
                        in_=xe[:, col0 + w - R : col0 + w],
                    )
                    pre.tensor_copy(
                        out=xe[:, col0 + w : xw], in_=xe[:, col0 : col0 + R]
                    )

                # y tiles are written once and stay resident all kernel
                y = work.tile([P, yw], bf16, tag=f"y{t}", bufs=1)
                # running-window recurrence: state = (xe[21+t] + state) - xe[t]
                nc.vector.tensor_tensor_scan(
                    out=y[:, 0:yw],
                    data0=xe[:, WIN : WIN + yw],
                    data1=xe[:, 0:yw],
                    initial=0.0,
                    op0=mybir.AluOpType.add,
                    op1=mybir.AluOpType.subtract,
                )
                y_tiles[t] = y

            def make_strip(r):
                """Output rows [128r, 128r + 128): matmuls + ACT drain."""
                y_cur = y_tiles[r]
                y_nxt = y_tiles[(r + 1) % nt]
                psum = psum_pool.tile([P, w], f32, tag="psum")
                for b in range(nbanks):
                    lo, hi = b * 512, min((b + 1) * 512, w)
                    nc.tensor.matmul(
                        psum[:, lo:hi],
                        lhsT=s1[:],
                        rhs=y_cur[:, 2 * R + lo : 2 * R + hi],
                        start=True,
                        stop=False,
                    )
                for b in range(nbanks):
                    lo, hi = b * 512, min((b + 1) * 512, w)
                    nc.tensor.matmul(
                        psum[:, lo:hi],
                        lhsT=s2[:],
                        rhs=y_nxt[:, 2 * R + lo : 2 * R + hi],
                        start=False,
                        stop=True,
                    )
                st = work.tile([P, w], f16, tag="st", bufs=ST_BUFS)
                if r == nt - 1:
                    # DVE is idle after the final scan; parallelize the two
                    # tail drains across DVE (strip 15) and ACT (strip 14)
                    nc.vector.tensor_copy(out=st[:], in_=psum[:])
                else:
                    nc.scalar.copy(st[:], psum[:])
                st_tiles[r] = st
                # even strips: ACT issues its own writeback right away
                if not (POOL_OUT and r % 2 == 1):
                    eng = nc.sync if r >= nt - 2 else nc.scalar
                    eng.dma_start(out=out[P * r : P * (r + 1), :], in_=st[:])

            def make_strip_out(r):
                """Odd strips drain on the Pool SWDGE ring, issued a few
                tiles late so the wait never stalls Pool's wrap copies."""
                if POOL_OUT and r % 2 == 1:
                    eng = nc.sync if r >= nt - 3 else nc.gpsimd
                    eng.dma_start(
                        out=out[P * r : P * (r + 1), :], in_=st_tiles[r][:]
                    )

            make_tile(0)
            for t in range(1, nt):
                make_tile(t)
                make_strip(t - 1)
                if t - 1 - OUT_LOOKAHEAD >= 0:
                    make_strip_out(t - 1 - OUT_LOOKAHEAD)
            make_strip(nt - 1)
            for r in range(nt - OUT_LOOKAHEAD - 1, nt):
                if r >= 0:
                    make_strip_out(r)

    nc.finalize()
    return nc


_BUILD_CACHE = {}


def _get_bass(h, w):
    key = (h, w, PRE_ENG, POOL_OUT, XE_BUFS, ST_BUFS, OUT_LOOKAHEAD)
    if key not in _BUILD_CACHE:
        _BUILD_CACHE[key] = _build_bass(h, w)
    return _BUILD_CACHE[key]


def _enable_ntff_tracing():
    """Harness-only: register the axon NTFF profile hook and stub the
    artifact upload (no bucket creds in this container)."""
    import antenv

    if not hasattr(antenv, "axon_hooks"):
        mod = types.ModuleType("antenv.axon_hooks")
        _hook = [None]
        mod.set_axon_ntff_profile_hook = lambda hk: _hook.__setitem__(0, hk)
        mod.get_axon_ntff_profile_hook = lambda: _hook[0]
        sys.modules["antenv.axon_hooks"] = mod
        antenv.axon_hooks = mod
    from trn_agent_boot.trn_boot import _ntff_profile_via_ctypes

    hook = _ntff_profile_via_ctypes("/opt/axon/libaxon_pjrt.so")
    if hook is not None:
        antenv.axon_hooks.set_axon_ntff_profile_hook(hook)
    bass_utils.upload_artifacts = lambda tmpdir: tmpdir


def run_hw(x, kernelx, trace=False):
    """Run the box filter on 8 NeuronCores. Returns (out, BassKernelResults)."""
    x = np.asarray(x)
    scale = float(np.asarray(kernelx).flat[0])
    s1, s2 = _band_matrices(scale)

    if trace:
        _enable_ntff_tracing()

    nc = _get_bass(H, W)
    xb = np.ascontiguousarray(x.astype(ml_dtypes.bfloat16))
    in_maps = [
        {"x": xb[i, 0], "s1": s1, "s2": s2} for i in range(B)
    ]
    r = bass_utils.run_bass_kernel_spmd(nc, in_maps, core_ids=list(range(B)),
                                        trace=trace)
    outs = np.stack([np.asarray(r.results[i]["out"]) for i in range(B)])[:, None]
    return outs.astype(np.float32), r


def _fallback_numpy(x, kernelx):
    """Exact (slow) path for a non-uniform kernel; never hit for the graded
    setup_inputs (all-ones kernel)."""
    x64 = np.asarray(x, dtype=np.float64)[:, 0]
    k = np.asarray(kernelx, dtype=np.float64)[0, 0]
    out = np.zeros_like(x64)
    for a in range(k.shape[0]):
        for b_ in range(k.shape[1]):
            if k[a, b_] == 0.0:
                continue
            out += k[a, b_] * np.roll(
                np.roll(x64, R - a, axis=1), R - b_, axis=2
            )
    return out[:, None].astype(np.float32)


def kernel(x, kernelx):
    kx = np.asarray(kernelx)
    if kx.size and not np.all(kx == kx.flat[0]):
        return _fallback_numpy(x, kernelx)
    out, _ = run_hw(x, kernelx, trace=False)
    return out
